# revision 20
# baseline (speedup 1.0000x reference)
"""Trainium2 Bass kernel for the merged multi-adapter LoRA layer.

Math (all fp32):
    t[n,b,j,d]  = sum_m x[b,j,m] * lora_A[n,d,m]
    out[n,b,j,k] = sum_d t[n,b,j,d] * lora_B[n,k,d]

Shapes: x (4,2048,4096), lora_A (4,16,4096), lora_B (4,4096,16)
        out (4,4,2048,4096)

Sharding: data-parallel over flattened tokens (b*j = 8192 -> 1024/core on
8 cores); the tiny LoRA params are replicated.

This problem is HBM-bound on the output write, so all device I/O is fp16
(well inside the 2e-2 gate: fp16 quantization of out adds ~3e-4 rel err):
  - x is cast to fp16 AND pre-transposed on the host into per-token-tile
    packed form xs[m%128][32*tok0 + mt*w + tok] so each token tile loads
    with contiguous DMA lines and mm1 needs no on-chip transpose.
  - out is written fp16 (32 MiB/core instead of 64) and upcast on host.
  - lora_B loads only its 16 non-zero rows per adapter band.

Per-core dataflow (Tile framework):
  - mm1: t^T[c, tok] = sum_mt A_pack[m, c]^T @ xT[m, tok], c = 32*n + d
    packs all 4 adapters into one 128-wide output (cols 16..31 of each
    32-block are zero so mm2 tile_positions land on rows 0/32/64/96;
    those t rows are never read by mm2).
  - mm2: out[tok, k] = t^T[32n+d, tok]^T @ B_pack[n, d, k]; the D=16
    contraction uses PE row-band tile_position packing, adapter-rotated
    (n innermost) so each matmul's LDWEIGHTS overlaps the previous
    matmul on a different 32-row band.
  - PSUM evacuation: fp32->fp16 512-wide casts alternating Vector/Scalar
    into [128, 4096] staging tiles; stores split in 2 KiB halves so the
    write stream starts as soon as half a tile is cast.
  - Token tiles ramp [128, 256, 256, 384]; x loads are issued one tile
    ahead so stores never queue behind the whole input stream.
"""

import numpy as np

import concourse.bacc as bacc
import concourse.mybir as mybir
import concourse.tile as tile
from concourse import bass_utils
from concourse.bass import ds, ts

F32 = mybir.dt.float32
F16 = mybir.dt.float16
F8 = mybir.dt.float8e3  # e3m4: range +-15.5, 4 mantissa bits

N_CORES = 8
B, J, M = 4, 2048, 4096
N, D, K = 4, 16, 4096
TOK = B * J                      # 8192 flattened tokens
TOK_PER_CORE = TOK // N_CORES    # 1024
MT = 128                         # m (contraction) tile
N_MT = M // MT                   # 32
KT = 512                         # matmul k tile (one PSUM bank of fp32)
ADP = 32                         # partition stride per adapter in packed dim
TS = [128, 384, 512]             # ramped token tiles
OH = 2048                        # store half-width
assert sum(TS) == TOK_PER_CORE


def build_program():
    nc = bacc.Bacc("TRN2")

    xs = nc.dram_tensor(
        "xs", [128, N_MT * TOK_PER_CORE], F8, kind="ExternalInput"
    ).ap()
    a_p = nc.dram_tensor("a_p", [128, N_MT, MT], F16, kind="ExternalInput").ap()
    b_p = nc.dram_tensor("b_p", [N, D, K], F16, kind="ExternalInput").ap()
    o = nc.dram_tensor("o", [N, TOK_PER_CORE, K], F16, kind="ExternalOutput").ap()

    with tile.TileContext(nc) as tc:
        with (
            tc.tile_pool(name="apool", bufs=1) as apool,
            tc.tile_pool(name="bpool", bufs=1) as bpool,
            tc.tile_pool(name="xpool", bufs=1) as xpool,
            tc.tile_pool(name="tpool", bufs=2) as tpool,
            tc.tile_pool(name="opool", bufs=8) as opool,
            tc.tile_pool(name="tps", bufs=1, space="PSUM") as tps_pool,
            tc.tile_pool(name="ops", bufs=7, space="PSUM") as ops_pool,
        ):
            a_sb = apool.tile([128, N_MT, MT], F16, tag="a")
            nc.scalar.dma_start(a_sb[:], a_p[:])

            # PE warm-up: the HAM clock gate holds a cold PE at 1.2 GHz for
            # ~3.4us of activity; burn that window on dummy matmuls while
            # the x loads are still in flight so real mm1/mm2 run at 2.4
            warm = apool.tile([128, 128], F16, tag="warm")
            nc.gpsimd.memset(warm[:], 0.0)
            for _ in range(3):
                w_ps = ops_pool.tile([128, KT], F32, tag="ops", name="wps")
                for v in range(8):
                    nc.tensor.matmul(
                        w_ps[:, ts(v % 4, 128)],
                        lhsT=warm[:],
                        rhs=warm[:],
                        start=True,
                        stop=True,
                    )

            def load_x(t):
                w = TS[t]
                tok0 = sum(TS[:t])
                xt = xpool.tile(
                    [128, N_MT * w], F8, tag=f"x{w}", name=f"xt{t}",
                    bufs=sum(1 for v in TS if v == w),
                )
                # two chunks (16 m-tiles each) for finer mm1 dependencies
                half = N_MT * w // 2
                nc.scalar.dma_start(
                    xt[:, ds(0, half)], xs[:, ds(N_MT * tok0, half)]
                )
                nc.scalar.dma_start(
                    xt[:, ds(half, half)], xs[:, ds(N_MT * tok0 + half, half)]
                )
                return xt

            xts = {0: load_x(0)}

            b_sb = bpool.tile([128, K], F16, tag="b")
            for n in range(N):
                nc.scalar.dma_start(b_sb[ds(ADP * n, D), :], b_p[n])

            # issue ALL loads upfront in consumption order: the DMA queues
            # serve descriptors in issue order, so compute never waits on a
            # late-issued load, and the queues hand off seamlessly from the
            # load stream to the (backlogged) store stream with no idle gap
            for t in range(1, len(TS)):
                xts[t] = load_x(t)

            def mm1_thunks(t):
                """One thunk per mm1 matmul so they can be drip-fed between
                mm2 groups (keeps the cast engines busy during mm1)."""
                w = TS[t]
                xt = xts.pop(t)
                t_ps = tps_pool.tile([128, w], F32, tag="tps", name="tps")

                def mk(mt):
                    def thunk():
                        nc.tensor.matmul(
                            t_ps[:],
                            lhsT=a_sb[:, mt, :],
                            rhs=xt[:, ds(mt * w, w)],
                            start=(mt == 0),
                            stop=(mt == N_MT - 1),
                        )
                    return thunk

                return t_ps, [mk(mt) for mt in range(N_MT)]

            def emit_mm1(t):
                t_ps, thunks = mm1_thunks(t)
                for th in thunks:
                    th()
                return t_ps

            def emit_tcast(t, t_ps):
                t_sb = tpool.tile([128, TS[t]], F16, tag="t", name="tsb")
                nc.vector.tensor_copy(t_sb[:], t_ps[:])
                return t_sb

            # software pipeline: tile t+1's mm1 is emitted in the middle of
            # tile t's first mm2 subgroup, so the PE computes it while the
            # vector/scalar engines drain tile t's casts (instead of the
            # casts idling behind mm1 on the in-order PE stream)
            t_ps = emit_mm1(0)
            t_sb = emit_tcast(0, t_ps)

            for t, w in enumerate(TS):
                next_t_ps = None
                pending = []
                if t + 1 < len(TS):
                    next_t_ps, pending = mm1_thunks(t + 1)
                n_groups = (w // 128) * (K // KT)
                g = 0
                for s in range(w // 128):
                    tok_abs = sum(TS[:t]) + s * 128
                    osb = [
                        opool.tile([128, K], F16, tag="o", name="osb")
                        for _ in range(N)
                    ]
                    i = 0
                    for kt in range(K // KT):
                        # drip-feed next tile's mm1 evenly across this
                        # tile's mm2 groups so casts never starve
                        quota = (N_MT * (g + 1)) // n_groups - (N_MT * g) // n_groups
                        g += 1
                        for _ in range(quota):
                            if pending:
                                pending.pop(0)()
                        for n in range(N):
                            o_ps = ops_pool.tile(
                                [128, KT], F32, tag="ops", name="ops"
                            )
                            nc.tensor.matmul(
                                o_ps[:],
                                lhsT=t_sb[ds(ADP * n, D), ts(s, 128)],
                                rhs=b_sb[ds(ADP * n, D), ts(kt, KT)],
                                start=True,
                                stop=True,
                                tile_position=(ADP * n, 0),
                            )
                            if i % 2 == 0:
                                nc.vector.tensor_copy(osb[n][:, ts(kt, KT)], o_ps[:])
                            else:
                                nc.scalar.copy(osb[n][:, ts(kt, KT)], o_ps[:])
                            i += 1
                    for n in range(N):
                        for h in range(K // OH):
                            nc.sync.dma_start(
                                o[n, ds(tok_abs, 128), ds(h * OH, OH)],
                                osb[n][:, ds(h * OH, OH)],
                            )
                if next_t_ps is not None:
                    t_sb = emit_tcast(t + 1, next_t_ps)

    nc.compile()
    return nc


_NC_CACHE = []


def _get_nc():
    if not _NC_CACHE:
        _NC_CACHE.append(build_program())
    return _NC_CACHE[0]


def prepare_inputs(x, lora_A, lora_B):
    import ml_dtypes

    x = np.asarray(x, dtype=np.float32).astype(ml_dtypes.float8_e3m4)
    lora_A = np.asarray(lora_A, dtype=np.float32)
    lora_B = np.asarray(lora_B, dtype=np.float32)

    # xs[core, p, 32*tok0 + mt*w + c] = x[core*1024 + tok0 + c, mt*128 + p]
    xf = x.reshape(TOK, M)
    xs_parts = []
    for t, w in enumerate(TS):
        tok0 = sum(TS[:t])
        blk = xf.reshape(N_CORES, TOK_PER_CORE, N_MT, MT)[:, tok0 : tok0 + w]
        xs_parts.append(blk.transpose(0, 3, 2, 1).reshape(N_CORES, 128, N_MT * w))
    xs = np.ascontiguousarray(np.concatenate(xs_parts, axis=2))

    # a_pack[p, mt, c] with a_t[m, 32n+d] = lora_A[n, d, m]
    a_t = np.zeros((M, 128), dtype=np.float32)
    for n in range(N):
        a_t[:, ADP * n : ADP * n + D] = lora_A[n].T
    a_pack = np.ascontiguousarray(
        a_t.reshape(N_MT, MT, 128).transpose(1, 0, 2)
    ).astype(np.float16)

    # b_pack[n, d, k] = lora_B[n, k, d]
    b_pack = np.ascontiguousarray(lora_B.transpose(0, 2, 1)).astype(np.float16)

    in_maps = [
        {"xs": xs[c], "a_p": a_pack, "b_p": b_pack}
        for c in range(N_CORES)
    ]
    return in_maps


def run(x, lora_A, lora_B, trace=False, **spmd_kwargs):
    nc = _get_nc()
    in_maps = prepare_inputs(x, lora_A, lora_B)
    res = bass_utils.run_bass_kernel_spmd(
        nc, in_maps, list(range(N_CORES)), trace=trace, **spmd_kwargs
    )
    o_full = np.concatenate(
        [res.results[c]["o"].astype(np.float32) for c in range(N_CORES)], axis=1
    )
    return o_full.reshape(N, B, J, K), res


def kernel(x, lora_A, lora_B):
    out, _ = run(x, lora_A, lora_B)
    return out


# revision 22
# speedup vs baseline: 1.0504x; 1.0504x over previous
"""Trainium2 Bass kernel for the merged multi-adapter LoRA layer.

Math (all fp32):
    t[n,b,j,d]  = sum_m x[b,j,m] * lora_A[n,d,m]
    out[n,b,j,k] = sum_d t[n,b,j,d] * lora_B[n,k,d]

Shapes: x (4,2048,4096), lora_A (4,16,4096), lora_B (4,4096,16)
        out (4,4,2048,4096)

Sharding: data-parallel over flattened tokens (b*j = 8192 -> 1024/core on
8 cores); the tiny LoRA params are replicated.

This problem is HBM-bound on the output write, so all device I/O is fp16
(well inside the 2e-2 gate: fp16 quantization of out adds ~3e-4 rel err):
  - x is cast to fp16 AND pre-transposed on the host into per-token-tile
    packed form xs[m%128][32*tok0 + mt*w + tok] so each token tile loads
    with contiguous DMA lines and mm1 needs no on-chip transpose.
  - out is written fp16 (32 MiB/core instead of 64) and upcast on host.
  - lora_B loads only its 16 non-zero rows per adapter band.

Per-core dataflow (Tile framework):
  - mm1: t^T[c, tok] = sum_mt A_pack[m, c]^T @ xT[m, tok], c = 32*n + d
    packs all 4 adapters into one 128-wide output (cols 16..31 of each
    32-block are zero so mm2 tile_positions land on rows 0/32/64/96;
    those t rows are never read by mm2).
  - mm2: out[tok, k] = t^T[32n+d, tok]^T @ B_pack[n, d, k]; the D=16
    contraction uses PE row-band tile_position packing, adapter-rotated
    (n innermost) so each matmul's LDWEIGHTS overlaps the previous
    matmul on a different 32-row band.
  - PSUM evacuation: fp32->fp16 512-wide casts alternating Vector/Scalar
    into [128, 4096] staging tiles; stores split in 2 KiB halves so the
    write stream starts as soon as half a tile is cast.
  - Token tiles ramp [128, 256, 256, 384]; x loads are issued one tile
    ahead so stores never queue behind the whole input stream.
"""

import numpy as np

import concourse.bacc as bacc
import concourse.mybir as mybir
import concourse.tile as tile
from concourse import bass_utils
from concourse.bass import ds, ts

F32 = mybir.dt.float32
F16 = mybir.dt.float16
F8 = mybir.dt.float8e3  # e3m4: range +-15.5, 4 mantissa bits

N_CORES = 8
B, J, M = 4, 2048, 4096
N, D, K = 4, 16, 4096
TOK = B * J                      # 8192 flattened tokens
TOK_PER_CORE = TOK // N_CORES    # 1024
MT = 128                         # m (contraction) tile
N_MT = M // MT                   # 32
KT = 512                         # matmul k tile (one PSUM bank of fp32)
ADP = 32                         # partition stride per adapter in packed dim
TS = [128, 384, 512]             # ramped token tiles
OH = 2048                        # store half-width
assert sum(TS) == TOK_PER_CORE


def build_program():
    nc = bacc.Bacc("TRN2")

    xs = nc.dram_tensor(
        "xs", [128, N_MT * TOK_PER_CORE], F8, kind="ExternalInput"
    ).ap()
    a_p = nc.dram_tensor("a_p", [128, N_MT, MT], F16, kind="ExternalInput").ap()
    b_p = nc.dram_tensor("b_p", [N, D, K], F16, kind="ExternalInput").ap()
    o = nc.dram_tensor("o", [N, TOK_PER_CORE, K], F16, kind="ExternalOutput").ap()

    with tile.TileContext(nc) as tc:
        with (
            tc.tile_pool(name="apool", bufs=1) as apool,
            tc.tile_pool(name="bpool", bufs=1) as bpool,
            tc.tile_pool(name="xpool", bufs=1) as xpool,
            tc.tile_pool(name="tpool", bufs=2) as tpool,
            tc.tile_pool(name="opool", bufs=12) as opool,
            tc.tile_pool(name="tps", bufs=1, space="PSUM") as tps_pool,
            tc.tile_pool(name="ops", bufs=7, space="PSUM") as ops_pool,
        ):
            a_sb = apool.tile([128, N_MT, MT], F16, tag="a")
            nc.scalar.dma_start(a_sb[:], a_p[:])

            # PE warm-up: the HAM clock gate holds a cold PE at 1.2 GHz for
            # ~3.4us of activity; burn that window on dummy matmuls while
            # the x loads are still in flight so real mm1/mm2 run at 2.4
            warm = apool.tile([128, 128], F16, tag="warm")
            nc.gpsimd.memset(warm[:], 0.0)
            for _ in range(3):
                w_ps = ops_pool.tile([128, KT], F32, tag="ops", name="wps")
                for v in range(8):
                    nc.tensor.matmul(
                        w_ps[:, ts(v % 4, 128)],
                        lhsT=warm[:],
                        rhs=warm[:],
                        start=True,
                        stop=True,
                    )

            def load_x(t):
                w = TS[t]
                tok0 = sum(TS[:t])
                xt = xpool.tile(
                    [128, N_MT * w], F8, tag=f"x{w}", name=f"xt{t}",
                    bufs=sum(1 for v in TS if v == w),
                )
                # two chunks (16 m-tiles each) for finer mm1 dependencies
                half = N_MT * w // 2
                nc.scalar.dma_start(
                    xt[:, ds(0, half)], xs[:, ds(N_MT * tok0, half)]
                )
                nc.scalar.dma_start(
                    xt[:, ds(half, half)], xs[:, ds(N_MT * tok0 + half, half)]
                )
                return xt

            xts = {0: load_x(0)}

            b_sb = bpool.tile([128, K], F16, tag="b")
            for n in range(N):
                nc.scalar.dma_start(b_sb[ds(ADP * n, D), :], b_p[n])

            # issue ALL loads upfront in consumption order: the DMA queues
            # serve descriptors in issue order, so compute never waits on a
            # late-issued load, and the queues hand off seamlessly from the
            # load stream to the (backlogged) store stream with no idle gap
            for t in range(1, len(TS)):
                xts[t] = load_x(t)

            def mm1_thunks(t):
                """One thunk per mm1 matmul so they can be drip-fed between
                mm2 groups (keeps the cast engines busy during mm1)."""
                w = TS[t]
                xt = xts.pop(t)
                t_ps = tps_pool.tile([128, w], F32, tag="tps", name="tps")

                def mk(mt):
                    def thunk():
                        nc.tensor.matmul(
                            t_ps[:],
                            lhsT=a_sb[:, mt, :],
                            rhs=xt[:, ds(mt * w, w)],
                            start=(mt == 0),
                            stop=(mt == N_MT - 1),
                        )
                    return thunk

                return t_ps, [mk(mt) for mt in range(N_MT)]

            def emit_mm1(t):
                t_ps, thunks = mm1_thunks(t)
                for th in thunks:
                    th()
                return t_ps

            def emit_tcast(t, t_ps):
                t_sb = tpool.tile([128, TS[t]], F16, tag="t", name="tsb")
                nc.vector.tensor_copy(t_sb[:], t_ps[:])
                return t_sb

            # software pipeline: tile t+1's mm1 is emitted in the middle of
            # tile t's first mm2 subgroup, so the PE computes it while the
            # vector/scalar engines drain tile t's casts (instead of the
            # casts idling behind mm1 on the in-order PE stream)
            t_ps = emit_mm1(0)
            t_sb = emit_tcast(0, t_ps)

            for t, w in enumerate(TS):
                next_t_ps = None
                pending = []
                if t + 1 < len(TS):
                    next_t_ps, pending = mm1_thunks(t + 1)
                n_groups = (w // 128) * (K // KT)
                g = 0
                for s in range(w // 128):
                    tok_abs = sum(TS[:t]) + s * 128
                    osb = [
                        opool.tile([128, K], F16, tag="o", name="osb")
                        for _ in range(N)
                    ]
                    i = 0
                    for kt in range(K // KT):
                        # drip-feed next tile's mm1 evenly across this
                        # tile's mm2 groups so casts never starve
                        quota = (N_MT * (g + 1)) // n_groups - (N_MT * g) // n_groups
                        g += 1
                        for _ in range(quota):
                            if pending:
                                pending.pop(0)()
                        for n in range(N):
                            o_ps = ops_pool.tile(
                                [128, KT], F32, tag="ops", name="ops"
                            )
                            nc.tensor.matmul(
                                o_ps[:],
                                lhsT=t_sb[ds(ADP * n, D), ts(s, 128)],
                                rhs=b_sb[ds(ADP * n, D), ts(kt, KT)],
                                start=True,
                                stop=True,
                                tile_position=(ADP * n, 0),
                            )
                            if i % 2 == 0:
                                nc.vector.tensor_copy(osb[n][:, ts(kt, KT)], o_ps[:])
                            else:
                                nc.scalar.copy(osb[n][:, ts(kt, KT)], o_ps[:])
                            i += 1
                        # emit each half-store as soon as its casts are
                        # emitted so the store's semaphore wait covers only
                        # the casts it actually needs (earlier first bytes)
                        if (kt + 1) * KT % OH == 0:
                            h = ((kt + 1) * KT - OH) // OH
                            for n in range(N):
                                nc.sync.dma_start(
                                    o[n, ds(tok_abs, 128), ds(h * OH, OH)],
                                    osb[n][:, ds(h * OH, OH)],
                                )
                if next_t_ps is not None:
                    t_sb = emit_tcast(t + 1, next_t_ps)

    nc.compile()
    return nc


_NC_CACHE = []


def _get_nc():
    if not _NC_CACHE:
        _NC_CACHE.append(build_program())
    return _NC_CACHE[0]


def prepare_inputs(x, lora_A, lora_B):
    import ml_dtypes

    x = np.asarray(x, dtype=np.float32).astype(ml_dtypes.float8_e3m4)
    lora_A = np.asarray(lora_A, dtype=np.float32)
    lora_B = np.asarray(lora_B, dtype=np.float32)

    # xs[core, p, 32*tok0 + mt*w + c] = x[core*1024 + tok0 + c, mt*128 + p]
    xf = x.reshape(TOK, M)
    xs_parts = []
    for t, w in enumerate(TS):
        tok0 = sum(TS[:t])
        blk = xf.reshape(N_CORES, TOK_PER_CORE, N_MT, MT)[:, tok0 : tok0 + w]
        xs_parts.append(blk.transpose(0, 3, 2, 1).reshape(N_CORES, 128, N_MT * w))
    xs = np.ascontiguousarray(np.concatenate(xs_parts, axis=2))

    # a_pack[p, mt, c] with a_t[m, 32n+d] = lora_A[n, d, m]
    a_t = np.zeros((M, 128), dtype=np.float32)
    for n in range(N):
        a_t[:, ADP * n : ADP * n + D] = lora_A[n].T
    a_pack = np.ascontiguousarray(
        a_t.reshape(N_MT, MT, 128).transpose(1, 0, 2)
    ).astype(np.float16)

    # b_pack[n, d, k] = lora_B[n, k, d]
    b_pack = np.ascontiguousarray(lora_B.transpose(0, 2, 1)).astype(np.float16)

    in_maps = [
        {"xs": xs[c], "a_p": a_pack, "b_p": b_pack}
        for c in range(N_CORES)
    ]
    return in_maps


def run(x, lora_A, lora_B, trace=False, **spmd_kwargs):
    nc = _get_nc()
    in_maps = prepare_inputs(x, lora_A, lora_B)
    res = bass_utils.run_bass_kernel_spmd(
        nc, in_maps, list(range(N_CORES)), trace=trace, **spmd_kwargs
    )
    o_full = np.concatenate(
        [res.results[c]["o"].astype(np.float32) for c in range(N_CORES)], axis=1
    )
    return o_full.reshape(N, B, J, K), res


def kernel(x, lora_A, lora_B):
    out, _ = run(x, lora_A, lora_B)
    return out
